# revision 1
# baseline (speedup 1.0000x reference)
"""Chamfer distance (B=8, N=M=4096, D=3) on 8 Trainium2 NeuronCores.

Strategy: data-parallel over batch — core b computes batch element b.

Per-core algorithm (one batch):
  The squared-distance matrix is produced NEGATED directly on the tensor
  engine via homogeneous coordinates, using FP32R (fast fp32 matmul mode,
  1 cycle/row vs 4 for fp32) with hi/lo splitting to recover full fp32
  accuracy.  Each scalar v is split as v = vh + vl (vh = fp32r-rounded),
  and products expand as x*y ~= xh*yh + xh*yl + xl*yh (the dropped xl*yl
  term is ~2^-22 relative).  With norm rows split the same way the
  augmented contraction has K=13 rows:
      a = (xh, xh, xl | nxh, nxl, 1, 1)        (coord triples)
      b = (2yh, 2yl, 2yh | -1, -1, -nyh, -nyl)
      sum_k a_k b_k = 2<x,y> - |x|^2 - |y|^2 = -d2[n, m]   (+ ~1e-5 err)
  One K=13 matmul per [128 x 512] block writes -d2 into PSUM.

  Per [128 x 2048] PSUM group (n-block i, m-group g):
    - ScalarE evicts PSUM -> SBUF as fp16 (`ev`).
    - DVE running max:  runs[g] = max(ev, runs[g])   (fp16, 2x mode)
        => column-wise max of -d2 over n  => dist2 side.
    - DVE fold tree over both groups of block i:
        fa = max(ev0, ev1); fb = max(fa_lo, fa_hi); ... ; reduce_max
        => clean row max of -d2 over all m  => dist1 side.

  Tail: dist1 from the fold accumulators; dist2 by transposing the
  running tiles through the PE and max-reducing.  relu (clamp at 0,
  commutes with min) and per-partition sums; the host sums the per-core
  [128, 2] partials and divides by B*N.
"""

import os
import sys

import numpy as np

for _p in ("/opt/trn_rl_repo", "/root/.axon_site/_ro/trn_rl_repo"):
    if os.path.isdir(_p) and _p not in sys.path:
        sys.path.append(_p)

B, N, M, D = 8, 4096, 4096, 3
P = 128
NCORES = 8
GW = 2048          # m-group width = 4 PSUM banks of fp32
NG = M // GW       # 2 m-groups
NB = N // P        # 32 n-blocks
MMF = 512          # matmul moving free dim (one PSUM bank)
KA = 13            # augmented contraction rows (hi/lo split)

_PROG = None


def _build_program(repeat: int = 1, variant: str = "full"):
    import concourse.mybir as mybir
    import concourse.tile as tile
    from concourse import bacc
    from concourse.masks import make_identity

    f32 = mybir.dt.float32
    f32r = mybir.dt.float32r
    f16 = mybir.dt.float16
    Alu = mybir.AluOpType
    Ax = mybir.AxisListType

    nc = bacc.Bacc("TRN2", target_bir_lowering=False, debug=False,
                   num_devices=NCORES)
    # Inputs reshaped host-side to [128, 96]: partition p holds points
    # 32p..32p+31 (3 floats each), contiguous per partition.
    x1d = nc.dram_tensor("xyz1", [P, N * D // P], f32, kind="ExternalInput").ap()
    x2d = nc.dram_tensor("xyz2", [P, M * D // P], f32, kind="ExternalInput").ap()
    outd = nc.dram_tensor("out", [P, 2], f32, kind="ExternalOutput").ap()

    with tile.TileContext(nc) as tc:
        with tc.tile_pool(name="persist", bufs=1) as persist:
            ident = persist.tile([P, P], f32)
            make_identity(nc, ident[:])
            identr = persist.tile([P, P], f32r)
            nc.vector.tensor_copy(identr[:], ident[:])
            ident16 = persist.tile([P, P], f16)
            nc.vector.tensor_copy(ident16[:], ident[:])

            X1 = persist.tile([P, 96], f32)
            X2 = persist.tile([P, 96], f32)
            nc.sync.dma_start(X1[:], x1d[:])
            nc.sync.dma_start(X2[:], x2d[:])

            ones2 = persist.tile([P, 64], f32)
            nc.vector.memset(ones2[:], 1.0)
            nones2 = persist.tile([P, 64], f32)
            nc.vector.memset(nones2[:], -1.0)

            def split_hi_lo(src_ap, shape, nm):
                """hi = fp32r(src), lo = fp32r(src - hi); returns (hi, lo)."""
                hi = persist.tile(shape, f32r, name=f"{nm}_hi")
                nc.vector.tensor_copy(hi[:], src_ap)
                lo32 = persist.tile(shape, f32, name=f"{nm}_lo32")
                nc.vector.tensor_sub(lo32[:], src_ap, hi[:].bitcast(f32))
                lo = persist.tile(shape, f32r, name=f"{nm}_lo")
                nc.vector.tensor_copy(lo[:], lo32[:])
                return hi, lo

            # --- input 1 (rows a): coords + norms
            X1h, X1l = split_hi_lo(X1[:], [P, 96], "x1")
            sq1 = persist.tile([P, 96], f32)
            nc.vector.tensor_mul(sq1[:], X1[:], X1[:])
            n1 = persist.tile([P, 32], f32)
            nc.vector.tensor_reduce(
                n1[:], sq1[:].rearrange("p (q d) -> p q d", d=D),
                axis=Ax.X, op=Alu.add)
            n1h, n1l = split_hi_lo(n1[:], [P, 32], "n1")

            # --- input 2 (rows b): scaled coords 2y + negated norms
            Y2 = persist.tile([P, 96], f32)
            nc.vector.tensor_scalar_mul(Y2[:], X2[:], 2.0)
            Y2h, Y2l = split_hi_lo(Y2[:], [P, 96], "y2")
            sq2 = persist.tile([P, 96], f32)
            nc.vector.tensor_mul(sq2[:], X2[:], X2[:])
            n2n = persist.tile([P, 32], f32)
            nc.vector.tensor_reduce(
                n2n[:], sq2[:].rearrange("p (q d) -> p q d", d=D),
                axis=Ax.X, op=Alu.add)
            nc.vector.tensor_scalar_mul(n2n[:], n2n[:], -1.0)
            n2h, n2l = split_hi_lo(n2n[:], [P, 32], "n2")

            # Interleaved augmented layout [128, 32*13], then transpose.
            A_t = persist.tile([P, 32 * KA], f32r)
            B_t = persist.tile([P, 32 * KA], f32r)
            Av = A_t[:].rearrange("p (q f) -> p q f", f=KA)
            Bv = B_t[:].rearrange("p (q f) -> p q f", f=KA)

            def v3(t):
                return t[:].rearrange("p (q d) -> p q d", d=D)

            def v1(t):
                return t[:].rearrange("p (q o) -> p q o", o=1)

            def v2(t):
                return t[:].rearrange("p (q o) -> p q o", o=2)

            # a rows: xh, xh, xl | nxh, nxl, 1, 1
            nc.vector.tensor_copy(Av[:, :, 0:3], v3(X1h))
            nc.vector.tensor_copy(Av[:, :, 3:6], v3(X1h))
            nc.vector.tensor_copy(Av[:, :, 6:9], v3(X1l))
            nc.vector.tensor_copy(Av[:, :, 9:10], v1(n1h))
            nc.vector.tensor_copy(Av[:, :, 10:11], v1(n1l))
            nc.vector.tensor_copy(Av[:, :, 11:13], v2(ones2))
            # b rows: 2yh, 2yl, 2yh | -1, -1, -nyh, -nyl
            nc.vector.tensor_copy(Bv[:, :, 0:3], v3(Y2h))
            nc.vector.tensor_copy(Bv[:, :, 3:6], v3(Y2l))
            nc.vector.tensor_copy(Bv[:, :, 6:9], v3(Y2h))
            nc.vector.tensor_copy(Bv[:, :, 9:11], v2(nones2))
            nc.vector.tensor_copy(Bv[:, :, 11:12], v1(n2h))
            nc.vector.tensor_copy(Bv[:, :, 12:13], v1(n2l))

            # Transpose to matmul layout: A5[k, n], B5[k, m] with
            # n-block q = points {32p + q}.  B first: it gates all matmuls.
            A5 = persist.tile([KA, N], f32r)
            B5 = persist.tile([KA, M], f32r)
            with tc.tile_pool(name="tpsum", bufs=4, space="PSUM") as tpsum:
                for src, dst in ((B_t, B5), (A_t, A5)):
                    for q in range(32):
                        pt = tpsum.tile([KA, P], f32r, tag="tp", name="pt")
                        nc.tensor.transpose(pt[:], src[:, KA * q:KA * (q + 1)],
                                            identr[:])
                        # alternate engines to halve the prologue wall time
                        if q % 2:
                            nc.scalar.copy(dst[:, P * q:P * (q + 1)], pt[:])
                        else:
                            nc.vector.tensor_copy(dst[:, P * q:P * (q + 1)], pt[:])

            # Running column-max of -d2 per m-group (=> min of d2 over n)
            runs = [persist.tile([P, GW], f16, name=f"run{g}") for g in range(NG)]
            # Clean per-(n-block, m-group) row-max accumulators (dist1 side)
            colmax = persist.tile([P, NB * NG], f32)

            with tc.tile_pool(name="mpsum", bufs=2, space="PSUM") as mpsum, \
                 tc.tile_pool(name="evp", bufs=3) as evp, \
                 tc.tile_pool(name="scrp", bufs=2) as scrp:
                for i in range(NB * repeat):
                    i = i % NB
                    for g in range(NG):
                        ps = mpsum.tile([P, GW], f32, tag="d2", name="ps")
                        for j in range(GW // MMF):
                            nc.tensor.matmul(
                                ps[:, MMF * j:MMF * (j + 1)],
                                lhsT=A5[:, P * i:P * (i + 1)],
                                rhs=B5[:, GW * g + MMF * j:GW * g + MMF * (j + 1)],
                                start=True, stop=True)
                        if variant == "pe":
                            continue
                        ev = evp.tile([P, GW], f16, tag="ev", name="ev")
                        nc.scalar.copy(ev[:], ps[:])
                        if variant == "evict":
                            continue
                        if variant in ("full", "ts"):
                            # dist1 side: row max (TS accum, 4x mode)
                            scr = scrp.tile([P, GW], f16, tag="scr", name="scr")
                            nc.vector.tensor_scalar(
                                out=scr[:], in0=ev[:], scalar1=-60000.0,
                                scalar2=None, op0=Alu.max, op1=Alu.max,
                                accum_out=colmax[:, NG * i + g:NG * i + g + 1])
                        if variant in ("full", "runs"):
                            # dist2 side: running column max (fp16, 2x mode)
                            if i == 0:
                                nc.vector.tensor_copy(runs[g][:], ev[:])
                            else:
                                nc.vector.tensor_max(runs[g][:], ev[:], runs[g][:])

            outsb = persist.tile([P, 2], f32)

            # dist1: combine group accumulators, clamp at 0, row-sum.
            d1 = persist.tile([P, NB], f32)
            cmv = colmax[:].rearrange("p (i g) -> p i g", g=NG)
            nc.vector.tensor_max(
                d1[:].rearrange("p (i o) -> p i o", o=1),
                cmv[:, :, 0:1], cmv[:, :, 1:2])
            d1r = persist.tile([P, NB], f32)
            nc.vector.tensor_scalar(
                out=d1r[:], in0=d1[:], scalar1=-1.0, scalar2=0.0,
                op0=Alu.mult, op1=Alu.max)
            nc.vector.tensor_reduce(outsb[:, 0:1], d1r[:], axis=Ax.X, op=Alu.add)

            # dist2: partition-axis max of running tiles via PE transpose.
            tmax = persist.tile([P, M // P], f32)
            with tc.tile_pool(name="tpsum2", bufs=4, space="PSUM") as tp2:
                for g in range(NG):
                    for c in range(GW // P):
                        pt2 = tp2.tile([P, P], f16, tag="tr", name="pt2")
                        nc.tensor.transpose(pt2[:], runs[g][:, P * c:P * (c + 1)],
                                            ident16[:])
                        col = (GW // P) * g + c
                        nc.vector.tensor_reduce(
                            tmax[:, col:col + 1], pt2[:], axis=Ax.X, op=Alu.max)
            d2r = persist.tile([P, M // P], f32)
            nc.vector.tensor_scalar(
                out=d2r[:], in0=tmax[:], scalar1=-1.0, scalar2=0.0,
                op0=Alu.mult, op1=Alu.max)
            nc.vector.tensor_reduce(outsb[:, 1:2], d2r[:], axis=Ax.X, op=Alu.add)

            nc.sync.dma_start(outd[:], outsb[:])

    nc.compile()
    return nc


def _get_program():
    global _PROG
    if _PROG is None:
        _PROG = _build_program()
    return _PROG


def kernel(xyz1: np.ndarray, xyz2: np.ndarray) -> np.ndarray:
    from concourse.bass_utils import run_bass_kernel_spmd

    xyz1 = np.asarray(xyz1, dtype=np.float32)
    xyz2 = np.asarray(xyz2, dtype=np.float32)
    assert xyz1.shape == (B, N, D) and xyz2.shape == (B, M, D)

    nc = _get_program()
    in_maps = [
        {
            "xyz1": np.ascontiguousarray(xyz1[b]).reshape(P, N * D // P),
            "xyz2": np.ascontiguousarray(xyz2[b]).reshape(P, M * D // P),
        }
        for b in range(NCORES)
    ]
    res = run_bass_kernel_spmd(nc, in_maps, list(range(NCORES))).results
    total = 0.0
    for r in res:
        total += float(r["out"].astype(np.float64).sum())
    # mean(dist1) + mean(dist2) = (sum dist1 + sum dist2) / (B*N)   (N == M)
    return np.float32(total / (B * N))



# revision 9
# speedup vs baseline: 1.2693x; 1.2693x over previous
"""Chamfer distance (B=8, N=M=4096, D=3) on 8 Trainium2 NeuronCores.

Strategy: data-parallel over batch — core b computes batch element b.

Per-core algorithm (one batch):
  The squared-distance matrix is produced NEGATED directly on the tensor
  engine via homogeneous coordinates, using FP32R (fast fp32 matmul mode,
  1 cycle/row vs 4 for fp32) with hi/lo splitting to recover full fp32
  accuracy.  Each scalar v is split as v = vh + vl (vh = fp32r-rounded),
  and products expand as x*y ~= xh*yh + xh*yl + xl*yh (the dropped xl*yl
  term is ~2^-22 relative).  With norm rows split the same way the
  augmented contraction has K=13 rows:
      a = (xh, xh, xl | nxh, nxl, 1, 1)        (coord triples)
      b = (2yh, 2yl, 2yh | -1, -1, -nyh, -nyl)
      sum_k a_k b_k = 2<x,y> - |x|^2 - |y|^2 = -d2[n, m]   (+ ~1e-5 err)
  One K=13 matmul per [128 x 512] block writes -d2 into PSUM.

  Per n-block i (two [128 x 2048] PSUM groups, 8 banks double-buffered):
    - ScalarE evicts both groups into one [128, 4096] fp16 tile `ev`.
    - DVE tensor_tensor_reduce folds the two halves (max) and
      max-reduces the fold => block row-max of -d2 => dist1 side,
      one 1x-mode pass instead of two.
    - Running column max (dist2 side) on DVE in fp16 2x mode, one wide
      [128, 4096] op per block.  (GPSIMD cannot run TensorTensor nor
      access PSUM on TRN2, so DVE owns both reduction passes.)

  Tail: dist1 from colmax accumulators (relu of negation, row-sum);
  dist2 by transposing the running tiles through the PE and
  max-reducing.  The host sums per-core [128, 2] partials / (B*N).
"""

import os
import sys

import numpy as np

for _p in ("/opt/trn_rl_repo", "/root/.axon_site/_ro/trn_rl_repo"):
    if os.path.isdir(_p) and _p not in sys.path:
        sys.path.append(_p)

B, N, M, D = 8, 4096, 4096, 3
P = 128
NCORES = 8
GW = 2048          # m-group width = 4 PSUM banks of fp32
NG = M // GW       # 2 m-groups
NB = N // P        # 32 n-blocks
MMF = 512          # matmul moving free dim (one PSUM bank)
KA = 13            # augmented contraction rows (hi/lo split)
SPLIT = 1152       # dist2 running-max m-split: [0,SPLIT) DVE, [SPLIT,M) GPSIMD

_PROG = None


def _build_program(repeat: int = 1, variant: str = "full"):
    import concourse.mybir as mybir
    import concourse.tile as tile
    from concourse import bacc
    from concourse.masks import make_identity

    f32 = mybir.dt.float32
    f32r = mybir.dt.float32r
    f16 = mybir.dt.float16
    Alu = mybir.AluOpType
    Ax = mybir.AxisListType

    nc = bacc.Bacc("TRN2", target_bir_lowering=False, debug=False,
                   num_devices=NCORES)
    # Inputs reshaped host-side to [128, 96]: partition p holds points
    # 32p..32p+31 (3 floats each), contiguous per partition.
    x1d = nc.dram_tensor("xyz1", [P, N * D // P], f32, kind="ExternalInput").ap()
    x2d = nc.dram_tensor("xyz2", [P, M * D // P], f32, kind="ExternalInput").ap()
    outd = nc.dram_tensor("out", [P, 2], f32, kind="ExternalOutput").ap()

    with tile.TileContext(nc) as tc:
        with tc.tile_pool(name="persist", bufs=1) as persist:
            ident = persist.tile([P, P], f32)
            make_identity(nc, ident[:])
            identr = persist.tile([P, P], f32r)
            nc.vector.tensor_copy(identr[:], ident[:])
            ident16 = persist.tile([P, P], f16)
            nc.vector.tensor_copy(ident16[:], ident[:])

            X1 = persist.tile([P, 96], f32)
            X2 = persist.tile([P, 96], f32)
            nc.sync.dma_start(X1[:], x1d[:])
            nc.sync.dma_start(X2[:], x2d[:])

            ones2 = persist.tile([P, 64], f32)
            nc.vector.memset(ones2[:], 1.0)
            nones2 = persist.tile([P, 64], f32)
            nc.vector.memset(nones2[:], -1.0)

            def split_hi_lo(src_ap, shape, nm):
                """hi = fp32r(src), lo = fp32r(src - hi); returns (hi, lo)."""
                hi = persist.tile(shape, f32r, name=f"{nm}_hi")
                nc.vector.tensor_copy(hi[:], src_ap)
                lo32 = persist.tile(shape, f32, name=f"{nm}_lo32")
                nc.vector.tensor_sub(lo32[:], src_ap, hi[:].bitcast(f32))
                lo = persist.tile(shape, f32r, name=f"{nm}_lo")
                nc.vector.tensor_copy(lo[:], lo32[:])
                return hi, lo

            # --- input 1 (rows a): coords + norms
            X1h, X1l = split_hi_lo(X1[:], [P, 96], "x1")
            sq1 = persist.tile([P, 96], f32)
            nc.vector.tensor_mul(sq1[:], X1[:], X1[:])
            n1 = persist.tile([P, 32], f32)
            nc.vector.tensor_reduce(
                n1[:], sq1[:].rearrange("p (q d) -> p q d", d=D),
                axis=Ax.X, op=Alu.add)
            n1h, n1l = split_hi_lo(n1[:], [P, 32], "n1")

            # --- input 2 (rows b): scaled coords 2y + negated norms
            Y2 = persist.tile([P, 96], f32)
            nc.vector.tensor_scalar_mul(Y2[:], X2[:], 2.0)
            Y2h, Y2l = split_hi_lo(Y2[:], [P, 96], "y2")
            sq2 = persist.tile([P, 96], f32)
            nc.vector.tensor_mul(sq2[:], X2[:], X2[:])
            n2n = persist.tile([P, 32], f32)
            nc.vector.tensor_reduce(
                n2n[:], sq2[:].rearrange("p (q d) -> p q d", d=D),
                axis=Ax.X, op=Alu.add)
            nc.vector.tensor_scalar_mul(n2n[:], n2n[:], -1.0)
            n2h, n2l = split_hi_lo(n2n[:], [P, 32], "n2")

            # Interleaved augmented layout [128, 32*13], then transpose.
            A_t = persist.tile([P, 32 * KA], f32r)
            B_t = persist.tile([P, 32 * KA], f32r)
            Av = A_t[:].rearrange("p (q f) -> p q f", f=KA)
            Bv = B_t[:].rearrange("p (q f) -> p q f", f=KA)

            def v3(t):
                return t[:].rearrange("p (q d) -> p q d", d=D)

            def v1(t):
                return t[:].rearrange("p (q o) -> p q o", o=1)

            def v2(t):
                return t[:].rearrange("p (q o) -> p q o", o=2)

            # a rows: xh, xh, xl | nxh, nxl, 1, 1
            nc.vector.tensor_copy(Av[:, :, 0:3], v3(X1h))
            nc.vector.tensor_copy(Av[:, :, 3:6], v3(X1h))
            nc.vector.tensor_copy(Av[:, :, 6:9], v3(X1l))
            nc.vector.tensor_copy(Av[:, :, 9:10], v1(n1h))
            nc.vector.tensor_copy(Av[:, :, 10:11], v1(n1l))
            nc.vector.tensor_copy(Av[:, :, 11:13], v2(ones2))
            # b rows: 2yh, 2yl, 2yh | -1, -1, -nyh, -nyl
            nc.vector.tensor_copy(Bv[:, :, 0:3], v3(Y2h))
            nc.vector.tensor_copy(Bv[:, :, 3:6], v3(Y2l))
            nc.vector.tensor_copy(Bv[:, :, 6:9], v3(Y2h))
            nc.vector.tensor_copy(Bv[:, :, 9:11], v2(nones2))
            nc.vector.tensor_copy(Bv[:, :, 11:12], v1(n2h))
            nc.vector.tensor_copy(Bv[:, :, 12:13], v1(n2l))

            # Transpose to matmul layout: A5[k, n], B5[k, m] with
            # n-block q = points {32p + q}.  B first: it gates all matmuls.
            A5 = persist.tile([KA, N], f32r)
            B5 = persist.tile([KA, M], f32r)
            with tc.tile_pool(name="tpsum", bufs=4, space="PSUM") as tpsum:
                for src, dst in ((B_t, B5), (A_t, A5)):
                    for q in range(32):
                        pt = tpsum.tile([KA, P], f32r, tag="tp", name="pt")
                        nc.tensor.transpose(pt[:], src[:, KA * q:KA * (q + 1)],
                                            identr[:])
                        # alternate engines to halve the prologue wall time
                        if q % 2:
                            nc.scalar.copy(dst[:, P * q:P * (q + 1)], pt[:])
                        else:
                            nc.vector.tensor_copy(dst[:, P * q:P * (q + 1)], pt[:])

            # dist2 running column-max of -d2 (=> min of d2 over n)
            runs = persist.tile([P, M], f16, name="runs")
            # dist1 per-block row-max accumulators (fp32, one col per block)
            colmax = persist.tile([P, NB], f32)
            nc.gpsimd.memset(runs[:], -60000.0)
            nc.gpsimd.memset(colmax[:], -60000.0)

            with tc.tile_pool(name="mpsum", bufs=2, space="PSUM") as mpsum, \
                 tc.tile_pool(name="evp", bufs=3) as evp, \
                 tc.tile_pool(name="scrp", bufs=2) as scrp:
                for i in range(NB * repeat):
                    i = i % NB
                    ev = evp.tile([P, M], f16, tag="ev", name="ev")
                    for g in range(NG):
                        ps = mpsum.tile([P, GW], f32, tag="d2", name="ps")
                        for j in range(GW // MMF):
                            nc.tensor.matmul(
                                ps[:, MMF * j:MMF * (j + 1)],
                                lhsT=A5[:, P * i:P * (i + 1)],
                                rhs=B5[:, GW * g + MMF * j:GW * g + MMF * (j + 1)],
                                start=True, stop=True)
                        if variant == "pe":
                            continue
                        nc.scalar.copy(ev[:, GW * g:GW * (g + 1)], ps[:])
                    if variant in ("pe", "evict"):
                        continue
                    if variant in ("full", "fold"):
                        # dist1: fold tree 4096 -> 512 (TT fp16 2x mode),
                        # then one narrow 1x reduce.
                        f1 = scrp.tile([P, 2048], f16, tag="f1", name="f1")
                        nc.vector.tensor_max(f1[:], ev[:, 0:2048],
                                             ev[:, 2048:4096])
                    if variant in ("full", "runs"):
                        # dist2: running column max (fp16 2x mode, one wide op)
                        if i == 0:
                            nc.vector.tensor_copy(runs[:], ev[:])
                        else:
                            nc.vector.tensor_max(runs[:], ev[:], runs[:])
                    if variant in ("full", "fold"):
                        f2 = scrp.tile([P, 1024], f16, tag="f2", name="f2")
                        nc.vector.tensor_max(f2[:], f1[:, 0:1024],
                                             f1[:, 1024:2048])
                        f3 = scrp.tile([P, 512], f16, tag="f3", name="f3")
                        nc.vector.tensor_max(f3[:], f2[:, 0:512],
                                             f2[:, 512:1024])
                        nc.vector.tensor_reduce(
                            colmax[:, i:i + 1], f3[:], axis=Ax.X, op=Alu.max)

            outsb = persist.tile([P, 2], f32)

            # dist1: clamp at 0 (relu of negation) and row-sum.
            d1r = persist.tile([P, NB], f32)
            nc.vector.tensor_scalar(
                out=d1r[:], in0=colmax[:], scalar1=-1.0, scalar2=0.0,
                op0=Alu.mult, op1=Alu.max)
            nc.vector.tensor_reduce(outsb[:, 0:1], d1r[:], axis=Ax.X, op=Alu.add)

            # dist2: partition-axis max of running tiles via PE transpose.
            tmax = persist.tile([P, M // P], f32)
            with tc.tile_pool(name="tpsum2", bufs=4, space="PSUM") as tp2:
                for c in range(M // P):
                    pt2 = tp2.tile([P, P], f16, tag="tr", name="pt2")
                    nc.tensor.transpose(pt2[:], runs[:, P * c:P * (c + 1)],
                                        ident16[:])
                    nc.vector.tensor_reduce(
                        tmax[:, c:c + 1], pt2[:], axis=Ax.X, op=Alu.max)
            d2r = persist.tile([P, M // P], f32)
            nc.vector.tensor_scalar(
                out=d2r[:], in0=tmax[:], scalar1=-1.0, scalar2=0.0,
                op0=Alu.mult, op1=Alu.max)
            nc.vector.tensor_reduce(outsb[:, 1:2], d2r[:], axis=Ax.X, op=Alu.add)

            nc.sync.dma_start(outd[:], outsb[:])

    nc.compile()
    return nc


def _get_program():
    global _PROG
    if _PROG is None:
        _PROG = _build_program()
    return _PROG


def kernel(xyz1: np.ndarray, xyz2: np.ndarray) -> np.ndarray:
    from concourse.bass_utils import run_bass_kernel_spmd

    xyz1 = np.asarray(xyz1, dtype=np.float32)
    xyz2 = np.asarray(xyz2, dtype=np.float32)
    assert xyz1.shape == (B, N, D) and xyz2.shape == (B, M, D)

    nc = _get_program()
    in_maps = [
        {
            "xyz1": np.ascontiguousarray(xyz1[b]).reshape(P, N * D // P),
            "xyz2": np.ascontiguousarray(xyz2[b]).reshape(P, M * D // P),
        }
        for b in range(NCORES)
    ]
    res = run_bass_kernel_spmd(nc, in_maps, list(range(NCORES))).results
    total = 0.0
    for r in res:
        total += float(r["out"].astype(np.float64).sum())
    # mean(dist1) + mean(dist2) = (sum dist1 + sum dist2) / (B*N)   (N == M)
    return np.float32(total / (B * N))
